# revision 15
# baseline (speedup 1.0000x reference)
"""AG-GEMM on 8 TRN2 NeuronCores.

Reference computes: A_full[8192, 4096] @ weight.T[4096, 4096] -> [8192, 4096],
where A_full is the concat of 8 per-rank shards A_shards[r] of [1024, 4096].

Strategy: pure row-parallel tensor parallelism. Core r computes
C_r = A_shards[r] @ weight.T with the full weight replicated per core, so no
collective is needed. Host pre-transposes both operands so the contraction
axis (K) lands on SBUF partitions:

  a blob per core  [128, 32*1024]: a[p, kt*1024+m] = A_r[m, kt*128+p]
  w blob (shared)  [32, 128, 4096]: w[nt, p, kt*128+j] = weight[nt*128+j, kt*128+p]

Per core the kernel keeps all of A resident in SBUF (16 MB), streams W
column-blocks (2 MB each, once), and accumulates C^T tiles in PSUM:

  out[nt, j, m] = sum_k w[k, nt*128+j] * a[k, m]   (C^T layout [4096, 1024])

Operands are converted to fp16 on the host: fp16 keeps tf32-grade precision
(10-bit mantissa, inputs are well-scaled randn) while streaming the PE at full
rate; measured end-to-end rel err vs the fp32 reference is ~1.9e-4. PSUM
accumulation stays fp32. W columns ride the ACT HWDGE ring and A tiles the SP
ring so the two streams don't serialize; W is chunked so the first matmul can
issue ~10us into the NEFF. Measured ~470us/core on silicon vs a 437us pure
PE-streaming floor.
"""

import numpy as np

WORLD = 8
M_LOCAL = 1024
K = 4096
N = 4096
KT = K // 128   # 32 k-tiles
NT = N // 128   # 32 n-tiles
MB = M_LOCAL // 512  # 2 moving blocks per k-tile

MM_DTYPE = "float16"  # 10-bit mantissa like tf32, full-rate PE, half DMA


def _enable_ldw_opt():
    """walrus's ldw-opt pass (disabled by default in concourse) splits
    self-loading matmuls into LDWEIGHTS+MATMUL so the weight load runs fast
    (FWL) and overlaps; safe for fp16 operands."""
    from concourse import bass_utils as bu

    if getattr(bu, "_ldw_opt_patched", False):
        return
    orig = bu.run_command

    def patched(cmd, *a, **kw):
        cmd = [
            c.replace("--enable-ldw-opt=false", "--enable-ldw-opt=true")
            if isinstance(c, str)
            else c
            for c in cmd
        ]
        return orig(cmd, *a, **kw)

    bu.run_command = patched
    bu._ldw_opt_patched = True


def _build_nc():
    from contextlib import ExitStack

    from concourse import bacc, mybir, tile


    f32 = mybir.dt.float32
    mm_dt = getattr(mybir.dt, MM_DTYPE)

    nc = bacc.Bacc("TRN2", target_bir_lowering=False, debug=False)

    a_ext = nc.dram_tensor("a", [128, KT * M_LOCAL], mm_dt, kind="ExternalInput")
    w_ext = nc.dram_tensor("w", [NT, 128, KT * 128], mm_dt, kind="ExternalInput")
    out_ext = nc.dram_tensor("out", [NT, 128, M_LOCAL], f32, kind="ExternalOutput")

    with tile.TileContext(nc) as tc, ExitStack() as ctx:
        a_pool = ctx.enter_context(tc.tile_pool(name="a", bufs=1))
        w_pool = ctx.enter_context(tc.tile_pool(name="w", bufs=3))
        o_pool = ctx.enter_context(tc.tile_pool(name="o", bufs=2))
        ps_pool = ctx.enter_context(tc.tile_pool(name="ps", bufs=4, space="PSUM"))

        def load_w(nt, nchunks=4):
            w_sb = w_pool.tile([128, KT * 128], mm_dt, name=f"w{nt}", tag="w")
            wc = KT * 128 // nchunks
            for c in range(nchunks):
                nc.scalar.dma_start(
                    w_sb[:, c * wc : (c + 1) * wc], w_ext[nt, :, c * wc : (c + 1) * wc]
                )
            return w_sb

        # Warm the PE (HAM un-throttle takes ~3.4us of sustained matmul
        # activity) with dummy matmuls on a zeroed tile while the first DMAs
        # are still in flight. Results land in a ps0-tag slot generation that
        # real columns later overwrite (start=True resets the bank).
        warm_pool = ctx.enter_context(tc.tile_pool(name="warm", bufs=1))
        warm_sb = warm_pool.tile([128, 640], mm_dt, name="warm_sb")
        nc.gpsimd.memset(warm_sb[:], 0.0)
        warm_ps = ps_pool.tile([128, 512], f32, name="warm_ps", tag="ps0")
        for _ in range(48):
            nc.tensor.matmul(
                warm_ps[:], warm_sb[:, :128], warm_sb[:, 128:640],
                start=True, stop=True,
            )

        # First W column before the A stream so the PE can start early;
        # W rides the ACT HWDGE ring, A the SP ring, so they don't serialize.
        w_next = load_w(0, nchunks=8)

        # A resident in SBUF, one DMA per k-tile so early matmuls don't
        # wait for the whole array.
        a_tiles = []
        for kt in range(KT):
            at = a_pool.tile([128, M_LOCAL], mm_dt, name=f"a{kt}", tag=f"a{kt}")
            nc.sync.dma_start(at[:], a_ext[:, kt * M_LOCAL : (kt + 1) * M_LOCAL])
            a_tiles.append(at)

        for nt in range(NT):
            w_sb = w_next
            if nt + 1 < NT:
                w_next = load_w(nt + 1)

            psums = [ps_pool.tile([128, 512], f32, name=f"ps{mb}", tag=f"ps{mb}") for mb in range(MB)]
            for kt in range(KT):
                lhsT = w_sb[:, kt * 128 : (kt + 1) * 128]
                for mb in range(MB):
                    nc.tensor.matmul(
                        psums[mb][:],
                        lhsT,
                        a_tiles[kt][:, mb * 512 : (mb + 1) * 512],
                        start=(kt == 0),
                        stop=(kt == KT - 1),
                    )

            o_sb = o_pool.tile([128, M_LOCAL], f32)
            for mb in range(MB):
                nc.vector.tensor_copy(o_sb[:, mb * 512 : (mb + 1) * 512], psums[mb][:])
                nc.sync.dma_start(
                    out_ext[nt, :, mb * 512 : (mb + 1) * 512],
                    o_sb[:, mb * 512 : (mb + 1) * 512],
                )

    nc.compile()
    return nc


def _round_tf32(x):
    """Round-to-nearest-even at 10-bit mantissa (TF32 grid) so fp32r HW
    rounding is a no-op on our values."""
    u = np.ascontiguousarray(x, dtype=np.float32).view(np.uint32)
    r = (u + np.uint32(0xFFF) + ((u >> np.uint32(13)) & np.uint32(1))) & np.uint32(0xFFFFE000)
    return r.view(np.float32)


def _prep_inputs(A_shards, weight, transed_weight=0):
    if MM_DTYPE == "float32r":
        A_shards = _round_tf32(A_shards)
        weight = _round_tf32(weight)
        np_dt = np.float32
    elif MM_DTYPE == "float16":
        np_dt = np.float16
    elif MM_DTYPE == "bfloat16":
        import ml_dtypes
        np_dt = ml_dtypes.bfloat16
    else:
        np_dt = np.float32
    A_shards = np.ascontiguousarray(A_shards, dtype=np_dt)
    weight = np.ascontiguousarray(weight, dtype=np_dt)

    try:
        transed = bool(int(np.asarray(transed_weight)))
    except (TypeError, ValueError):
        transed = bool(transed_weight)

    # w blob: [nt, p, kt*128+j] = W[kt*128+p, nt*128+j] where W is [K, N]
    if transed:
        # weight is already [K, N]
        w_blob = np.ascontiguousarray(
            weight.reshape(KT, 128, NT, 128).transpose(2, 1, 0, 3).reshape(NT, 128, KT * 128)
        )
    else:
        # weight is [N, K]; W = weight.T -> blob[nt,p,kt*128+j] = weight[nt*128+j, kt*128+p]
        w_blob = np.ascontiguousarray(
            weight.reshape(NT, 128, KT, 128).transpose(0, 3, 2, 1).reshape(NT, 128, KT * 128)
        )

    in_maps = []
    for r in range(WORLD):
        # a blob: [p, kt*1024+m] = A_r[m, kt*128+p]
        a_blob = np.ascontiguousarray(
            A_shards[r].T.reshape(KT, 128, M_LOCAL).transpose(1, 0, 2).reshape(128, KT * M_LOCAL)
        )
        in_maps.append({"a": a_blob, "w": w_blob})
    return in_maps


def _gather_output(results):
    # per-core out [NT, 128, M_LOCAL] is C_r^T tiles: out[nt, j, m] = C_r[m, nt*128+j]
    parts = []
    for r in range(WORLD):
        o = results[r]["out"]
        parts.append(o.transpose(2, 0, 1).reshape(M_LOCAL, N))
    return np.ascontiguousarray(np.concatenate(parts, axis=0))


_NC = None


def _get_nc():
    global _NC
    if _NC is None:
        _NC = _build_nc()
    return _NC


def kernel(A_shards, weight, transed_weight=0, **_ignored):
    from concourse import bass_utils

    nc = _get_nc()
    in_maps = _prep_inputs(A_shards, weight, transed_weight)
    res = bass_utils.run_bass_kernel_spmd(nc, in_maps, core_ids=list(range(WORLD)))
    return _gather_output(res.results)


if __name__ == "__main__":
    rng = np.random.default_rng(0)
    A = rng.standard_normal((WORLD, M_LOCAL, K), dtype=np.float32)
    W = (rng.standard_normal((N, K), dtype=np.float32) * 0.02).astype(np.float32)
    out = kernel(A, W, 0)
    ref = A.reshape(WORLD * M_LOCAL, K) @ W.T
    err = np.abs(out - ref).max() / max(np.abs(ref).max(), 1e-12)
    print("abs-rel err vs local numpy:", err)


# revision 16
# speedup vs baseline: 1.0241x; 1.0241x over previous
"""AG-GEMM on 8 TRN2 NeuronCores.

Reference computes: A_full[8192, 4096] @ weight.T[4096, 4096] -> [8192, 4096],
where A_full is the concat of 8 per-rank shards A_shards[r] of [1024, 4096].

Strategy: pure row-parallel tensor parallelism. Core r computes
C_r = A_shards[r] @ weight.T with the full weight replicated per core, so no
collective is needed. Host pre-transposes both operands so the contraction
axis (K) lands on SBUF partitions:

  a blob per core  [128, 32*1024]: a[p, kt*1024+m] = A_r[m, kt*128+p]
  w blob (shared)  [32, 128, 4096]: w[nt, p, kt*128+j] = weight[nt*128+j, kt*128+p]

Per core the kernel keeps all of A resident in SBUF (16 MB), streams W
column-blocks (2 MB each, once), and accumulates C^T tiles in PSUM:

  out[nt, j, m] = sum_k w[k, nt*128+j] * a[k, m]   (C^T layout [4096, 1024])

Operands are converted to fp16 on the host: fp16 keeps tf32-grade precision
(10-bit mantissa, inputs are well-scaled randn) while streaming the PE at full
rate; measured end-to-end rel err vs the fp32 reference is ~1.9e-4. PSUM
accumulation stays fp32. W columns ride the ACT HWDGE ring and A tiles the SP
ring so the two streams don't serialize; W is chunked so the first matmul can
issue ~10us into the NEFF. Measured ~470us/core on silicon vs a 437us pure
PE-streaming floor.
"""

import numpy as np

WORLD = 8
M_LOCAL = 1024
K = 4096
N = 4096
KT = K // 128   # 32 k-tiles
NT = N // 128   # 32 n-tiles
MB = M_LOCAL // 512  # 2 moving blocks per k-tile

MM_DTYPE = "float16"  # 10-bit mantissa like tf32, full-rate PE, half DMA


def _enable_ldw_opt():
    """walrus's ldw-opt pass (disabled by default in concourse) splits
    self-loading matmuls into LDWEIGHTS+MATMUL so the weight load runs fast
    (FWL) and overlaps; safe for fp16 operands."""
    from concourse import bass_utils as bu

    if getattr(bu, "_ldw_opt_patched", False):
        return
    orig = bu.run_command

    def patched(cmd, *a, **kw):
        cmd = [
            c.replace("--enable-ldw-opt=false", "--enable-ldw-opt=true")
            if isinstance(c, str)
            else c
            for c in cmd
        ]
        return orig(cmd, *a, **kw)

    bu.run_command = patched
    bu._ldw_opt_patched = True


def _build_nc():
    from contextlib import ExitStack

    from concourse import bacc, mybir, tile


    f32 = mybir.dt.float32
    mm_dt = getattr(mybir.dt, MM_DTYPE)

    nc = bacc.Bacc("TRN2", target_bir_lowering=False, debug=False)

    a_ext = nc.dram_tensor("a", [128, KT * M_LOCAL], mm_dt, kind="ExternalInput")
    w_ext = nc.dram_tensor("w", [NT, 128, KT * 128], mm_dt, kind="ExternalInput")
    out_ext = nc.dram_tensor("out", [NT, 128, M_LOCAL], f32, kind="ExternalOutput")

    with tile.TileContext(nc) as tc, ExitStack() as ctx:
        a_pool = ctx.enter_context(tc.tile_pool(name="a", bufs=1))
        w_pool = ctx.enter_context(tc.tile_pool(name="w", bufs=3))
        o_pool = ctx.enter_context(tc.tile_pool(name="o", bufs=2))
        ps_pool = ctx.enter_context(tc.tile_pool(name="ps", bufs=4, space="PSUM"))

        def load_w(nt, nchunks=4):
            w_sb = w_pool.tile([128, KT * 128], mm_dt, name=f"w{nt}", tag="w")
            wc = KT * 128 // nchunks
            for c in range(nchunks):
                nc.scalar.dma_start(
                    w_sb[:, c * wc : (c + 1) * wc], w_ext[nt, :, c * wc : (c + 1) * wc]
                )
            return w_sb

        # First W column before the A stream so the PE can start early;
        # W rides the ACT HWDGE ring, A the SP ring, so they don't serialize.
        w_next = load_w(0, nchunks=8)

        # A resident in SBUF, one DMA per k-tile so early matmuls don't
        # wait for the whole array.
        a_tiles = []
        for kt in range(KT):
            at = a_pool.tile([128, M_LOCAL], mm_dt, name=f"a{kt}", tag=f"a{kt}")
            nc.sync.dma_start(at[:], a_ext[:, kt * M_LOCAL : (kt + 1) * M_LOCAL])
            a_tiles.append(at)

        for nt in range(NT):
            w_sb = w_next
            if nt + 1 < NT:
                w_next = load_w(nt + 1)

            psums = [ps_pool.tile([128, 512], f32, name=f"ps{mb}", tag=f"ps{mb}") for mb in range(MB)]
            for kt in range(KT):
                lhsT = w_sb[:, kt * 128 : (kt + 1) * 128]
                for mb in range(MB):
                    nc.tensor.matmul(
                        psums[mb][:],
                        lhsT,
                        a_tiles[kt][:, mb * 512 : (mb + 1) * 512],
                        start=(kt == 0),
                        stop=(kt == KT - 1),
                    )

            o_sb = o_pool.tile([128, M_LOCAL], f32)
            for mb in range(MB):
                nc.vector.tensor_copy(o_sb[:, mb * 512 : (mb + 1) * 512], psums[mb][:])
                nc.sync.dma_start(
                    out_ext[nt, :, mb * 512 : (mb + 1) * 512],
                    o_sb[:, mb * 512 : (mb + 1) * 512],
                )

    nc.compile()
    return nc


def _round_tf32(x):
    """Round-to-nearest-even at 10-bit mantissa (TF32 grid) so fp32r HW
    rounding is a no-op on our values."""
    u = np.ascontiguousarray(x, dtype=np.float32).view(np.uint32)
    r = (u + np.uint32(0xFFF) + ((u >> np.uint32(13)) & np.uint32(1))) & np.uint32(0xFFFFE000)
    return r.view(np.float32)


def _prep_inputs(A_shards, weight, transed_weight=0):
    if MM_DTYPE == "float32r":
        A_shards = _round_tf32(A_shards)
        weight = _round_tf32(weight)
        np_dt = np.float32
    elif MM_DTYPE == "float16":
        np_dt = np.float16
    elif MM_DTYPE == "bfloat16":
        import ml_dtypes
        np_dt = ml_dtypes.bfloat16
    else:
        np_dt = np.float32
    A_shards = np.ascontiguousarray(A_shards, dtype=np_dt)
    weight = np.ascontiguousarray(weight, dtype=np_dt)

    try:
        transed = bool(int(np.asarray(transed_weight)))
    except (TypeError, ValueError):
        transed = bool(transed_weight)

    # w blob: [nt, p, kt*128+j] = W[kt*128+p, nt*128+j] where W is [K, N]
    if transed:
        # weight is already [K, N]
        w_blob = np.ascontiguousarray(
            weight.reshape(KT, 128, NT, 128).transpose(2, 1, 0, 3).reshape(NT, 128, KT * 128)
        )
    else:
        # weight is [N, K]; W = weight.T -> blob[nt,p,kt*128+j] = weight[nt*128+j, kt*128+p]
        w_blob = np.ascontiguousarray(
            weight.reshape(NT, 128, KT, 128).transpose(0, 3, 2, 1).reshape(NT, 128, KT * 128)
        )

    in_maps = []
    for r in range(WORLD):
        # a blob: [p, kt*1024+m] = A_r[m, kt*128+p]
        a_blob = np.ascontiguousarray(
            A_shards[r].T.reshape(KT, 128, M_LOCAL).transpose(1, 0, 2).reshape(128, KT * M_LOCAL)
        )
        in_maps.append({"a": a_blob, "w": w_blob})
    return in_maps


def _gather_output(results):
    # per-core out [NT, 128, M_LOCAL] is C_r^T tiles: out[nt, j, m] = C_r[m, nt*128+j]
    parts = []
    for r in range(WORLD):
        o = results[r]["out"]
        parts.append(o.transpose(2, 0, 1).reshape(M_LOCAL, N))
    return np.ascontiguousarray(np.concatenate(parts, axis=0))


_NC = None


def _get_nc():
    global _NC
    if _NC is None:
        _NC = _build_nc()
    return _NC


def kernel(A_shards, weight, transed_weight=0, **_ignored):
    from concourse import bass_utils

    nc = _get_nc()
    in_maps = _prep_inputs(A_shards, weight, transed_weight)
    res = bass_utils.run_bass_kernel_spmd(nc, in_maps, core_ids=list(range(WORLD)))
    return _gather_output(res.results)


if __name__ == "__main__":
    rng = np.random.default_rng(0)
    A = rng.standard_normal((WORLD, M_LOCAL, K), dtype=np.float32)
    W = (rng.standard_normal((N, K), dtype=np.float32) * 0.02).astype(np.float32)
    out = kernel(A, W, 0)
    ref = A.reshape(WORLD * M_LOCAL, K) @ W.T
    err = np.abs(out - ref).max() / max(np.abs(ref).max(), 1e-12)
    print("abs-rel err vs local numpy:", err)
